# revision 1
# baseline (speedup 1.0000x reference)
"""Paged GQA attention Bass kernel for TRN2, SPMD over 8 cores (v4).

Sharding: tensor-parallel over KV heads. Core h owns KV head h and its 4
query heads. Per-core: B=4 seqs x S=2048 kv x (4 heads * 256 q) x d=128.

v4/oxt data path — the paged indirection is resolved with ONE block-granular
gather per seq (the v3 per-token gathers cost ~95us/rep on HW):
  - rep 0 (one-time): the per-head cache slice [2, 640, 16*128] f32 HBM is
    cast-DMA'd to an HBM bf16 blob tensor kvb [641, 4096]: blob b =
    [K block b (16 slots x 128 d) | V block b]. Blob 640 is zeroed (pad).
    q is cast-staged to SBUF qcb bf16 (as in v3).
  - per seq: dma_gather (transpose=True, num_idxs=128 = 112 real blocks +
    16 pads at blob 640, elem 8KB) -> kvt [128, 32, 128] where
    kvt[p, c, j] = blob(bt[j])[c*128+p]: c<16 is K^T tile c (partition=d),
    c>=16 is V^T. The bulk kv position order is PERMUTED (pos = 16j + c) —
    attention is permutation-invariant as long as K columns, V rows and
    masks agree, and bulk positions 0..1791 are fully visible (no masks).
  - V^T bulk -> V via one XBAR dma_start_transpose ([128,2048] -> 16 tiles).
    Pad rows have V=0 so they never pollute PV; their exp(0)=1 rows are
    removed from the denominator by an exact -256 correction.
  - tail (the Q=256 new tokens, positions 1792..2047): dense k/v loads ->
    bf16 casts (Pool) -> K^T via XBAR; 2 causally-masked tiles as in v3.
  - q^T per seq via 2 XBAR transposes from qcb.
  - compute per seq: 18 tiles: scores^T = K_tile^T.T @ q^T -> exp (ACT) ->
    optional mask (DVE) -> partial += et (DVE) -> PV accumulate (PE, one
    tile behind). finalize: den = ones @ partial - 256, reciprocal,
    XBAR output transpose + normalize into full 2KB rows, 2 out DMAs/seq.
"""
import numpy as np
import ml_dtypes

import concourse.bass as bass
import concourse.bacc as bacc
import concourse.mybir as mybir
from concourse.tile import TileContext

F32 = mybir.dt.float32
BF16 = mybir.dt.bfloat16
I16 = mybir.dt.int16

B, Q, S = 4, 256, 2048
G, D = 4, 128
BLOCK = 16
NBULK = 112              # bulk blocks per seq (positions 0..1791)
NBLK = 640               # cache pool blocks
ZBLK = NBLK              # zero (pad) blob id
QW = G * Q               # 1024
NBT = 16                 # bulk tiles per seq
NTT = 2                  # tail tiles per seq
NT = NBT + NTT           # 18
SCALE = float(D) ** -0.5
PAD_DEN = float(16 * NBT)  # pad rows (16 partitions) x bulk tiles, exp(0)=1


def build_masks(seq_lens, tail_inorder=False):
    """Masks for the v4 tile structure: 16 bulk tiles (permuted positions,
    pad rows always visible) + 2 tail tiles (positions 1792..2047)."""
    mask_arrays, needs, cache = {}, {}, {}

    def add(key, vis):
        if vis.all():
            needs[key] = None
            return
        k = vis.tobytes()
        if k not in cache:
            name = f"mask{len(cache)}"
            cache[k] = name
            mask_arrays[name] = vis.astype(ml_dtypes.bfloat16)
        needs[key] = cache[k]

    for b in range(B):
        sl = int(seq_lens[b])
        qpos = sl - Q + (np.arange(QW) % Q)
        for i in range(NBT):
            j = np.arange(128)
            kpos = 16 * j + i
            pad = j >= NBULK
            vis = ((kpos[:, None] <= qpos[None, :])
                   & (kpos[:, None] < sl)) | pad[:, None]
            add((b, i), vis)
        for t in range(NTT):
            if tail_inorder:
                kpos = S - Q + t * 128 + np.arange(128)
            else:
                kpos = S - Q + 2 * np.arange(128) + t
            vis = (kpos[:, None] <= qpos[None, :]) & (kpos[:, None] < sl)
            add((b, NBT + t), vis)
    return mask_arrays, needs


def build_nc(seq_lens=(2048,) * B, variant="full", repeat=1):
    nc = bacc.Bacc(None, target_bir_lowering=False, debug=False)

    q_ext = nc.declare_dram_parameter("q", [B * Q, G * D], F32, isOutput=False)
    k_ext = nc.declare_dram_parameter("k", [B * Q, D], F32, isOutput=False)
    v_ext = nc.declare_dram_parameter("v", [B * Q, D], F32, isOutput=False)
    kvc_ext = nc.declare_dram_parameter("kvc", [2, NBLK, BLOCK * D], F32,
                                        isOutput=False)
    btw_ext = nc.declare_dram_parameter("btw", [128, B * 8], I16, isOutput=False)
    idb_ext = nc.declare_dram_parameter("idb", [128, 128], BF16, isOutput=False)
    idf_ext = nc.declare_dram_parameter("idf", [128, 128], F32, isOutput=False)
    ones_ext = nc.declare_dram_parameter("onesb", [128, 1], BF16, isOutput=False)

    mask_arrays, mask_needs = build_masks(seq_lens, tail_inorder=True)
    mask_ext = {
        name: nc.declare_dram_parameter(name, [128, QW], BF16, isOutput=False)
        for name in mask_arrays
    }

    out_ext = nc.declare_dram_parameter("out", [B * Q, G * D], F32, isOutput=True)

    from contextlib import ExitStack

    with TileContext(nc) as tc, ExitStack() as stack:
        cpool = stack.enter_context(tc.tile_pool(name="consts", bufs=1))
        dpool = stack.enter_context(tc.tile_pool(name="dram", bufs=1, space="DRAM"))
        spool = stack.enter_context(tc.tile_pool(name="sbuf", bufs=3))
        idxpool = stack.enter_context(tc.tile_pool(name="idxp", bufs=2))
        et_pool = stack.enter_context(tc.tile_pool(name="et", bufs=8))
        ppool_sc = stack.enter_context(tc.tile_pool(name="psc", bufs=2, space="PSUM"))
        ppool_o = stack.enter_context(tc.tile_pool(name="po", bufs=1, space="PSUM"))
        ppool_t = stack.enter_context(tc.tile_pool(name="pt", bufs=2, space="PSUM"))

        # ---- constants ----
        idb = cpool.tile([128, 128], BF16, tag="idb")
        nc.sync.dma_start(out=idb[:], in_=idb_ext[:, :])
        idf = cpool.tile([128, 128], F32, tag="idf")
        nc.sync.dma_start(out=idf[:], in_=idf_ext[:, :])
        onesb = cpool.tile([128, 1], BF16, tag="onesb")
        nc.sync.dma_start(out=onesb[:], in_=ones_ext[:, :])
        masks = {}
        for name in mask_ext:
            m = cpool.tile([128, QW], BF16, tag=name)
            nc.sync.dma_start(out=m[:], in_=mask_ext[name][:, :])
            masks[name] = m

        if variant == "noop":
            z = spool.tile([128, 128], F32, tag="outsb")
            nc.vector.memset(z[:], 0.0)
            nc.sync.dma_start(out=out_ext[0:128, 0:128], in_=z[:])

        # PE clock warm-up (HAM gate holds PE at 1.2 GHz until ~3.4us busy).
        if variant != "noop":
            warm = ppool_t.tile([128, 128], F32, tag="tp", name="warm")
            for _w in range(28):
                nc.tensor.matmul(warm[:], lhsT=idb[:], rhs=idb[:],
                                 start=True, stop=True)

        # ---- one-time staging ----
        # kvb blob tensor: [K blob | V blob] per block, bf16, + zero blob.
        kvb = dpool.tile([NBLK + 1, 2 * BLOCK * D], BF16, tag="kvb")
        nc.gpsimd.dma_start(
            out=kvb[0:NBLK, :].rearrange("b (k e) -> k b e", k=2, e=BLOCK * D),
            in_=kvc_ext[:, :, :],
        )
        zt = cpool.tile([128, 32], BF16, tag="zt")
        nc.vector.memset(zt[:], 0.0)
        nc.sync.dma_start(
            out=kvb[NBLK : NBLK + 1, :].rearrange("a (p c) -> p (a c)", p=128),
            in_=zt[:],
        )
        # q staged bf16: [p=tok%128, r=tok//128, (h d)]
        qcb = cpool.tile([128, (B * Q // 128) * G * D], BF16, tag="qcb")
        qcb_v = qcb[:].rearrange("p (r hd) -> p r hd", r=B * Q // 128, hd=G * D)
        nc.gpsimd.dma_start(
            out=qcb_v[:, :, :],
            in_=q_ext.rearrange("(r p) hd -> p r hd", p=128),
        )

        def emit_prep_dma(b, btwsb, pair=False):
            st = {}
            nj = 256 if pair else 128
            st["kvt"] = spool.tile([128, 32 * nj], BF16, tag="kvt", name="kvt")
            nc.gpsimd.dma_gather(
                out_ap=st["kvt"][:].rearrange("p (c j) -> p c j", c=32, j=nj),
                in_ap=kvb[:, :],
                idxs_ap=btwsb[:, b * 8 : b * 8 + nj // 16],
                num_idxs=nj, num_idxs_reg=nj, elem_size=2 * BLOCK * D,
                transpose=True, single_packet=(variant == "sp1"),
            )
            emit_prep_tails(b, st)
            return st

        def emit_prep_tails(b, st):
            tail_arr = "(t p) d -> p t d"
            ldeng = nc.gpsimd if variant == "poolq" else nc.sync
            st["ktl"] = spool.tile([128, 2 * D], F32, tag="ktl", name="ktl")
            ldeng.dma_start(
                out=st["ktl"][:].rearrange("p (t d) -> p t d", t=2, d=D),
                in_=k_ext[b * Q : (b + 1) * Q, :].rearrange(
                    tail_arr, t=2, p=128
                ),
            )
            st["vtl"] = spool.tile([128, 2 * D], F32, tag="vtl", name="vtl")
            ldeng.dma_start(
                out=st["vtl"][:].rearrange("p (t d) -> p t d", t=2, d=D),
                in_=v_ext[b * Q : (b + 1) * Q, :].rearrange(
                    tail_arr, t=2, p=128
                ),
            )
            return st

        def emit_prep_compute(b, st, pair_side=None):
            nj = 128 if pair_side is None else 256
            kvt_v = st["kvt"][:].rearrange("p (c j) -> p c j", c=32, j=nj)
            # V bulk: V^T [128 d, shat] -> v-tiles [128 shat, d]
            if pair_side is None:
                vtb = spool.tile([128, NBT * D], BF16, tag="vtb")
                vtb_v = vtb[:].rearrange("p (c d) -> p c d", c=NBT, d=D)
                nc.sync.dma_start_transpose(
                    out=vtb_v, in_=st["kvt"][:, 16 * nj : 32 * nj],
                )
                st["kt_tiles"] = [kvt_v[:, i, :] for i in range(NBT)]
                st["v_tiles"] = [vtb_v[:, i, :] for i in range(NBT)]
            else:
                sd = pair_side
                if sd == 0:
                    vtbp = spool.tile([128, 2 * NBT * D], BF16, tag="vtb")
                    nc.sync.dma_start_transpose(
                        out=vtbp[:].rearrange("p (m d) -> p m d",
                                              m=2 * NBT, d=D),
                        in_=st["kvt"][:, 16 * nj : 32 * nj],
                    )
                    st["vtbp"] = vtbp
                vtb_v = st["vtbp"][:].rearrange(
                    "p (c s d) -> p c s d", c=NBT, s=2, d=D)
                st["kt_tiles"] = [kvt_v[:, i, sd * 128 : (sd + 1) * 128]
                                  for i in range(NBT)]
                st["v_tiles"] = [vtb_v[:, i, sd, :] for i in range(NBT)]
            # tails: cast f32->bf16 on Pool; K tail XBAR-transposed
            ktlb = spool.tile([128, 2 * D], BF16, tag="ktlb")
            nc.gpsimd.tensor_copy(ktlb[:], st["ktl"][:])
            ktail = spool.tile([128, 2 * D], BF16, tag="ktail")
            nc.sync.dma_start_transpose(
                out=ktail[:].rearrange("p (t d) -> p t d", t=2, d=D),
                in_=ktlb[:],
            )
            vtail = spool.tile([128, 2 * D], BF16, tag="vtail")
            nc.gpsimd.tensor_copy(vtail[:], st["vtl"][:])
            for t in range(NTT):
                st["kt_tiles"].append(ktail[:, t * D : (t + 1) * D])
                st["v_tiles"].append(vtail[:, t * D : (t + 1) * D])
            # q^T: [128 d, (h, t, tok)] via 2 XBAR transposes
            qt_t = spool.tile([128, QW], BF16, tag="qt")
            qt_v = qt_t[:].rearrange("p (h t k) -> p h t k", h=G, t=2, k=128)
            for t in range(2):
                nc.sync.dma_start_transpose(
                    out=qt_v[:, :, t, :],
                    in_=qcb_v[:, 2 * b + t, :],
                )
            st["qt"] = qt_t
            st["kvt_v"] = kvt_v

        def emit_compute(b, st, mid_hook=None):
            partial = spool.tile([128, QW], BF16, tag="partial")
            psum_o = ppool_o.tile([128, QW], F32, tag="po")
            qt_t = st["qt"]
            kt_tiles, v_tiles = st["kt_tiles"], st["v_tiles"]

            def emit_pv(i, et):
                if variant == "nopv":
                    if i == 0:
                        nc.tensor.matmul(
                            psum_o[:, 0:512], lhsT=v_tiles[0],
                            rhs=et[:, 0:512], start=True, stop=True,
                        )
                    return
                v_tile = v_tiles[i]
                for half in range(2):
                    nc.tensor.matmul(
                        psum_o[:, half * 512 : (half + 1) * 512],
                        lhsT=v_tile,
                        rhs=et[:, half * 512 : (half + 1) * 512],
                        start=(i == 0), stop=(i == NT - 1),
                    )

            pv_lag = 2 if variant == "pv2" else 1
            ets = {}
            for i in range(NT):
                if i == 6 and mid_hook is not None:
                    mid_hook()
                kt_cols = kt_tiles[i]
                psc = ppool_sc.tile([128, QW], F32, tag="psc")
                for half in range(2):
                    nc.tensor.matmul(
                        psc[:, half * 512 : (half + 1) * 512],
                        lhsT=kt_cols,
                        rhs=qt_t[:, half * 512 : (half + 1) * 512],
                        start=True, stop=True,
                    )
                et = et_pool.tile([128, QW], BF16, tag="et")
                if variant == "noexp":
                    nc.scalar.activation(
                        et[:, 0:128], psc[:, 0:128],
                        mybir.ActivationFunctionType.Exp, scale=SCALE,
                    )
                else:
                    nc.scalar.activation(
                        et[:], psc[:], mybir.ActivationFunctionType.Exp, scale=SCALE
                    )
                mname = mask_needs[(b, i)]
                if mname is not None:
                    nc.vector.tensor_mul(et[:], et[:], masks[mname][:])
                if i == 0:
                    nc.vector.tensor_copy(partial[:], et[:])
                elif variant != "nodve":
                    nc.vector.tensor_add(partial[:], partial[:], et[:])
                ets[i] = et
                if i - pv_lag >= 0:
                    emit_pv(i - pv_lag, ets.pop(i - pv_lag))
            for i in range(NT - pv_lag, NT):
                emit_pv(i, ets.pop(i))
            osb = spool.tile([128, QW], BF16, tag="osb")
            nc.vector.tensor_copy(osb[:], psum_o[:])
            st["partial"], st["osb"] = partial, osb

        def emit_finalize(b, st):
            partial, osb = st["partial"], st["osb"]
            den_ps = ppool_sc.tile([1, QW], F32, tag="psc")
            for half in range(2):
                nc.tensor.matmul(
                    den_ps[:, half * 512 : (half + 1) * 512],
                    lhsT=onesb[:],
                    rhs=partial[:, half * 512 : (half + 1) * 512],
                    start=True, stop=True,
                )
            den_sb = spool.tile([1, QW], F32, tag="densb")
            # exact removal of the pad rows' exp(0)=1 contributions
            nc.vector.tensor_scalar(
                out=den_sb[:], in0=den_ps[:], scalar1=-PAD_DEN, scalar2=None,
                op0=mybir.AluOpType.add,
            )
            rp_ps = ppool_t.tile([128, 8], F32, tag="tp")
            for j in range(8):
                nc.tensor.transpose(
                    rp_ps[:, j : j + 1], den_sb[0:1, j * 128 : (j + 1) * 128],
                    idf[0:1, 0:1],
                )
            rp_sb = spool.tile([128, 8], F32, tag="rpsb")
            nc.vector.tensor_copy(rp_sb[:], rp_ps[:])
            recip = spool.tile([128, 8], F32, tag="recip")
            nc.vector.reciprocal(recip[:], rp_sb[:])

            if variant not in ("pet", "osmall"):
                # assemble full 2KB output rows in SBUF -> 2 DMAs/seq with
                # 2KB descriptors instead of 8 DMAs of 512B column slices
                ot = spool.tile([128, QW], BF16, tag="ot")
                nc.sync.dma_start_transpose(
                    out=ot[:].rearrange("p (j d) -> p j d", j=8, d=D),
                    in_=osb[:],
                )
                for tt in range(2):
                    of = spool.tile([128, G * D], F32, tag="outf")
                    for h in range(G):
                        j = h * 2 + tt
                        nc.vector.tensor_scalar(
                            out=of[:, h * D : (h + 1) * D],
                            in0=ot[:, j * 128 : (j + 1) * 128],
                            scalar1=recip[:, j : j + 1],
                            scalar2=None, op0=mybir.AluOpType.mult,
                        )
                    (nc.gpsimd if variant == "poolq" else nc.sync).dma_start(
                        out=out_ext[b * Q + tt * 128 : b * Q + (tt + 1) * 128, :],
                        in_=of[:],
                    )
            elif variant == "osmall":
                ot = spool.tile([128, QW], BF16, tag="ot")
                nc.sync.dma_start_transpose(
                    out=ot[:].rearrange("p (j d) -> p j d", j=8, d=D),
                    in_=osb[:],
                )
                for j in range(8):
                    o_sb = spool.tile([128, 128], F32, tag="outsb")
                    nc.vector.tensor_scalar(
                        out=o_sb[:], in0=ot[:, j * 128 : (j + 1) * 128],
                        scalar1=recip[:, j : j + 1],
                        scalar2=None, op0=mybir.AluOpType.mult,
                    )
                    h, tt = j // 2, j % 2
                    nc.sync.dma_start(
                        out=out_ext[
                            b * Q + tt * 128 : b * Q + (tt + 1) * 128,
                            h * D : (h + 1) * D,
                        ],
                        in_=o_sb[:],
                    )
            else:
                for j in range(8):
                    ps = ppool_t.tile([128, 128], BF16, tag="tp")
                    nc.tensor.transpose(ps[:], osb[:, j * 128 : (j + 1) * 128], idb[:])
                    o_sb = spool.tile([128, 128], F32, tag="outsb")
                    nc.vector.tensor_scalar(
                        out=o_sb[:], in0=ps[:], scalar1=recip[:, j : j + 1],
                        scalar2=None, op0=mybir.AluOpType.mult,
                    )
                    h, tt = j // 2, j % 2
                    nc.sync.dma_start(
                        out=out_ext[
                            b * Q + tt * 128 : b * Q + (tt + 1) * 128,
                            h * D : (h + 1) * D,
                        ],
                        in_=o_sb[:],
                    )

        for _rep in range(repeat if variant != "noop" else 0):
            btwsb = idxpool.tile([128, B * 8], I16, tag="btwsb")
            (nc.gpsimd if variant == "poolq" else nc.sync).dma_start(
                out=btwsb[:], in_=btw_ext[:, :])
            if variant != "seqgather":
                st = {}
                st[0] = emit_prep_dma(0, btwsb, pair=True)
                st[1] = {"kvt": st[0]["kvt"]}
                emit_prep_tails(1, st[1])
                emit_prep_compute(0, st[0], pair_side=0)
                st[1]["vtbp"] = st[0]["vtbp"]
                for b in range(B):
                    if b == 1:
                        st[2] = emit_prep_dma(2, btwsb, pair=True)
                        st[3] = {"kvt": st[2]["kvt"]}
                        emit_prep_tails(3, st[3])
                    if b - 1 >= 0:
                        fb = b - 1
                        hook = (lambda fb=fb: (emit_finalize(fb, st[fb]),
                                               st.pop(fb)))
                    else:
                        hook = None
                    emit_compute(b, st[b], mid_hook=hook)
                    if b + 1 < B:
                        emit_prep_compute(b + 1, st[b + 1],
                                          pair_side=(b + 1) % 2)
                        if (b + 1) % 2 == 0:
                            st[b + 2]["vtbp"] = st[b + 1]["vtbp"]
                emit_finalize(B - 1, st[B - 1])
            else:
                st = {0: emit_prep_dma(0, btwsb)}
                emit_prep_compute(0, st[0])
                for b in range(B):
                    if b + 1 < B:
                        st[b + 1] = emit_prep_dma(b + 1, btwsb)
                    if b - 1 >= 0:
                        fb = b - 1
                        hook = lambda fb=fb: (emit_finalize(fb, st[fb]),
                                              st.pop(fb))
                    else:
                        hook = None
                    emit_compute(b, st[b], mid_hook=hook)
                    if b + 1 < B:
                        emit_prep_compute(b + 1, st[b + 1])
                emit_finalize(B - 1, st[B - 1])

    nc.finalize()
    return nc, mask_arrays


def make_consts():
    idb = np.eye(128).astype(ml_dtypes.bfloat16)
    idf = np.eye(128, dtype=np.float32)
    onesb = np.ones((128, 1), ml_dtypes.bfloat16)
    return dict(idb=idb, idf=idf, onesb=onesb)


def check_invariant(slot_mapping, block_tables):
    pos = np.arange(S - Q, S)
    want = (block_tables[:, pos // BLOCK] * BLOCK + pos % BLOCK).reshape(-1)
    return np.array_equal(slot_mapping.reshape(-1), want)


def shard_inputs(q, k, v, kv_cache, slot_mapping, block_tables, seq_lens,
                 query_start_loc, mask_arrays):
    consts = make_consts()
    kv_cache = np.asarray(kv_cache)
    block_tables = np.asarray(block_tables)
    k_use, v_use = k, v
    kc_all = kv_cache[0]          # [640, 16, 8, 128]
    vc_all = kv_cache[1]
    if not check_invariant(slot_mapping, block_tables):
        kc_all = kc_all.copy().reshape(NBLK * BLOCK, 8, D)
        vc_all = vc_all.copy().reshape(NBLK * BLOCK, 8, D)
        sm = np.asarray(slot_mapping).reshape(-1)
        kc_all[sm] = np.asarray(k).reshape(-1, 8, D)
        vc_all[sm] = np.asarray(v).reshape(-1, 8, D)
        pos = np.arange(S - Q, S)
        slots_b = (block_tables[:, pos // BLOCK] * BLOCK + pos % BLOCK).reshape(-1)
        k_use = kc_all[slots_b].reshape(B * Q, 8 * D)
        v_use = vc_all[slots_b].reshape(B * Q, 8 * D)
        kc_all = kc_all.reshape(NBLK, BLOCK, 8, D)
        vc_all = vc_all.reshape(NBLK, BLOCK, 8, D)
    # gather index tile [128, B*8]: wrapped in 16 partitions, replicated
    btw = np.zeros((128, B * 8), np.int16)
    for b in range(B):
        bt_pad = np.concatenate(
            [block_tables[b, :NBULK], np.full(16, ZBLK)]).astype(np.int16)
        for p in range(128):
            for c in range(8):
                btw[p, b * 8 + c] = bt_pad[c * 16 + p % 16]
    in_maps = []
    for h in range(8):
        kvc = np.stack([
            np.ascontiguousarray(kc_all[:, :, h, :]).reshape(NBLK, BLOCK * D),
            np.ascontiguousarray(vc_all[:, :, h, :]).reshape(NBLK, BLOCK * D),
        ])
        m = {
            "q": np.ascontiguousarray(q[:, h * G * D : (h + 1) * G * D]),
            "k": np.ascontiguousarray(k_use[:, h * D : (h + 1) * D]),
            "v": np.ascontiguousarray(v_use[:, h * D : (h + 1) * D]),
            "kvc": kvc,
            "btw": btw,
            **consts,
            **mask_arrays,
        }
        in_maps.append(m)
    return in_maps


def assemble_output(results):
    return np.concatenate([results[h]["out"] for h in range(8)], axis=1)


# ---------------------------------------------------------------------------
# Harness entry point: kernel(**inputs) with FULL (unsharded) inputs.
# ---------------------------------------------------------------------------
from concourse.bass_utils import run_bass_kernel_spmd

_CACHE = {}


def _get_nc(seq_lens):
    key = tuple(int(x) for x in seq_lens)
    if key not in _CACHE:
        _CACHE[key] = build_nc(key)
    return _CACHE[key]


def kernel(q, k, v, kv_cache, slot_mapping, block_tables, seq_lens,
           query_start_loc, **extra):
    q = np.asarray(q); k = np.asarray(k); v = np.asarray(v)
    kv_cache = np.asarray(kv_cache)
    slot_mapping = np.asarray(slot_mapping)
    block_tables = np.asarray(block_tables)
    seq_lens = np.asarray(seq_lens)
    nc, mask_arrays = _get_nc(seq_lens)
    in_maps = shard_inputs(q, k, v, kv_cache, slot_mapping, block_tables,
                           seq_lens, query_start_loc, mask_arrays)
    res = run_bass_kernel_spmd(nc, in_maps, core_ids=list(range(8)))
    return assemble_output(res.results)

